# revision 1
# baseline (speedup 1.0000x reference)
"""Trainium2 Bass kernel for windowed multi-head attention (ClassicAttention).

Shapes (hardcoded per spec): x (1024, 68, 768), pe (128, 768), mask zeros.
Data-parallel over 8 NeuronCores on the leading window axis.
"""

import os
import sys

for _p in (
    "/root/.axon_site",
    "/root/.axon_site/_ro/trn_rl_repo",
    "/root/.axon_site/_ro/pypackages",
    "/opt/trn_rl_repo",
):
    if os.path.isdir(_p) and _p not in sys.path:
        sys.path.append(_p)

import numpy as np

import concourse.bass as bass  # noqa: F401  (engine types referenced via nc)
import concourse.mybir as mybir
import concourse.tile as tile
from concourse import bacc
from concourse.bass_utils import run_bass_kernel_spmd

F32 = mybir.dt.float32
AFT = mybir.ActivationFunctionType

NCORES = 8
B_, N, C = 1024, 68, 768
H, HD = 12, 64
N_VTS = 4
KT = C // 128            # 6 contraction tiles of 128
BL = B_ // NCORES        # 128 windows per core
G = 4                    # windows per group (fp32 moving-operand 4*68=272 <= 512)
NG = BL // G             # 32 groups
FD = G * N               # 272

_CACHE = {}
DEBUG_TAPS = False


def _build_nc(mm_dt=F32):
    nc = bacc.Bacc(trn_type="TRN2", target_bir_lowering=False, debug=False)
    dbg = {}
    if DEBUG_TAPS:
        dbg["qkt"] = nc.dram_tensor("dbg_qkt", [128, 12, G, N], F32, kind="ExternalOutput")
        dbg["v"] = nc.dram_tensor("dbg_v", [N, G, C], F32, kind="ExternalOutput")
        dbg["esn"] = nc.dram_tensor("dbg_esn", [N, H, N], F32, kind="ExternalOutput")
        dbg["at"] = nc.dram_tensor("dbg_at", [128, KT, G, N], F32, kind="ExternalOutput")

    xt_d = nc.dram_tensor("xt", [128, KT, BL, N], F32, kind="ExternalInput")
    w1_d = nc.dram_tensor("w1", [128, 12, KT, 128], F32, kind="ExternalInput")
    w2_d = nc.dram_tensor("w2", [128, KT, C], F32, kind="ExternalInput")
    wp_d = nc.dram_tensor("wp", [128, KT, KT, 128], F32, kind="ExternalInput")
    peqk_d = nc.dram_tensor("peqk", [N, 12, 128], F32, kind="ExternalInput")
    pev_d = nc.dram_tensor("pev", [N, C], F32, kind="ExternalInput")
    bp_d = nc.dram_tensor("bp", [1, C], F32, kind="ExternalInput")
    i68r_d = nc.dram_tensor("i68r", [N, FD], F32, kind="ExternalInput")
    ones1_d = nc.dram_tensor("ones1", [1, FD], F32, kind="ExternalInput")
    ones68_d = nc.dram_tensor("ones68", [N, 1], F32, kind="ExternalInput")
    out_d = nc.dram_tensor("outt", [128, KT, BL, N], F32, kind="ExternalOutput")

    RDT = mm_dt                       # dtype of big-GEMM operands
    cast_dma = nc.gpsimd if mm_dt != F32 else nc.sync

    def mm(ap):
        return ap

    with tile.TileContext(nc) as tc:
        with (
            tc.tile_pool(name="wgt", bufs=1) as wp_pool,
            tc.tile_pool(name="xp", bufs=2) as xp,
            tc.tile_pool(name="qkp", bufs=2) as qkp,
            tc.tile_pool(name="vp", bufs=2) as vp,
            tc.tile_pool(name="esp", bufs=2) as esp,
            tc.tile_pool(name="atp", bufs=2) as atp,
            tc.tile_pool(name="rp", bufs=2) as rp,
            tc.tile_pool(name="pbig", bufs=2, space="PSUM") as pbig,
            tc.tile_pool(name="ppv", bufs=1, space="PSUM") as ppv,
            tc.tile_pool(name="psc", bufs=2, space="PSUM") as psc,
            tc.tile_pool(name="ps1", bufs=1, space="PSUM") as ps1p,
            tc.tile_pool(name="pav", bufs=1, space="PSUM") as pavp,
        ):
            W1s = wp_pool.tile([128, 12, KT, 128], RDT)
            W2s = wp_pool.tile([128, KT, C], RDT)
            WPs = wp_pool.tile([128, KT, KT, 128], RDT)
            PEQKs = wp_pool.tile([N, 12, 128], RDT)
            PEVs = wp_pool.tile([N, C], RDT)
            BPs = wp_pool.tile([1, C], RDT)
            I68Rs = wp_pool.tile([N, FD], RDT)
            ONES1s = wp_pool.tile([1, FD], RDT)
            ONES68s = wp_pool.tile([N, 1], F32)
            cast_dma.dma_start(W1s[:], w1_d.ap())
            cast_dma.dma_start(W2s[:], w2_d.ap())
            cast_dma.dma_start(WPs[:], wp_d.ap())
            cast_dma.dma_start(PEQKs[:], peqk_d.ap())
            cast_dma.dma_start(PEVs[:], pev_d.ap())
            cast_dma.dma_start(BPs[:], bp_d.ap())
            cast_dma.dma_start(I68Rs[:], i68r_d.ap())
            cast_dma.dma_start(ONES1s[:], ones1_d.ap())
            nc.sync.dma_start(ONES68s[:], ones68_d.ap())

            for g in range(NG):
                gsl = slice(G * g, G * (g + 1))
                XT = xp.tile([128, KT, G, N], RDT, tag="xt")
                cast_dma.dma_start(XT[:], xt_d.ap()[:, :, gsl, :])

                # ---- q,k in transposed layout: QKT[p, j, w, t] (j<6: q, j>=6: k)
                QKT = qkp.tile([128, 12, G, N], F32, tag="qkt")
                for j in range(12):
                    pq = pbig.tile([128, FD], F32, tag="big")
                    for k in range(KT):
                        nc.tensor.matmul(
                            pq[:], mm(W1s[:, j, k, :]), mm(XT[:, k, :, :]),
                            start=(k == 0), stop=False,
                        )
                    nc.tensor.matmul(
                        pq[:], mm(PEQKs[:, j, :]), mm(I68Rs[:]),
                        start=False, stop=True,
                    )
                    dst = QKT[:, j, :, :]
                    src = pq.rearrange("p (a b) -> p a b", a=G)
                    if j % 2 == 0:
                        nc.vector.tensor_copy(dst, src)
                    else:
                        nc.scalar.copy(dst, src)

                # ---- v in direct layout: V[t, w, o]
                V = vp.tile([N, G, C], F32, tag="v")
                for w in range(G):
                    pv = ppv.tile([N, C], F32, tag="pv")
                    for hs in (slice(0, 512), slice(512, C)):
                        for k in range(KT):
                            nc.tensor.matmul(
                                pv[:, hs], mm(XT[:, k, w, :]), mm(W2s[:, k, hs]),
                                start=(k == 0), stop=False,
                            )
                        nc.tensor.matmul(
                            pv[:, hs], mm(I68Rs[:, :N]), mm(PEVs[:, hs]),
                            start=False, stop=True,
                        )
                    nc.scalar.copy(V[:, w, :], pv[:])
                if DEBUG_TAPS and g == 0:
                    nc.sync.dma_start(dbg["qkt"].ap(), QKT[:])
                    nc.sync.dma_start(dbg["v"].ap(), V[:])

                AT = atp.tile([128, KT, G, N], RDT, tag="at")

                def qk_softmax(w):
                    # ES slot s = 6*half + hh holds head h = 2*hh + half, so
                    # each PSUM bank sees a single PE row-group (HW hangs on
                    # mixed-row-group matmuls into one bank).
                    ES = esp.tile([N, H, N], F32, tag="es")
                    R1 = rp.tile([1, 2, 408], F32, tag="r1")
                    for half in range(2):
                        sc = psc.tile([N, 6, N], F32, tag="sc")
                        p0 = 64 * half
                        for hh in range(6):
                            nc.tensor.matmul(
                                sc[:, hh, :],
                                mm(QKT[p0:p0 + 64, 6 + hh, w, :]),
                                mm(QKT[p0:p0 + 64, hh, w, :]),
                                start=True, stop=True, skip_group_check=True,
                            )
                        nc.scalar.activation(
                            ES[:, 6 * half:6 * half + 6, :], sc[:], AFT.Exp
                        )
                        s1 = ps1p.tile([1, 408], F32, tag="s1")
                        nc.tensor.matmul(
                            s1[:], mm(ONES68s[:]),
                            mm(ES[:, 6 * half:6 * half + 6, :]),
                            start=True, stop=True,
                        )
                        nc.vector.reciprocal(R1[:, half, :], s1[:])
                    RBS = esp.tile([N, H * N], F32, tag="rbs")
                    r1f = R1.rearrange("p a b -> p (a b)")
                    src = bass.AP(
                        tensor=r1f.tensor, offset=r1f.offset,
                        ap=[list(r1f.ap[0]), [0, N], [1, H * N]],
                    )
                    nc.sync.dma_start(RBS[:], src)
                    ESN = esp.tile([N, H, N], F32, tag="esn")
                    nc.vector.tensor_mul(
                        ESN[:], ES[:], RBS.rearrange("p (a b) -> p a b", a=H)
                    )
                    if DEBUG_TAPS and g == 0 and w == 0:
                        nc.sync.dma_start(dbg["esn"].ap(), ESN[:])
                    return ESN

                def av(w, ESN):
                    pa = pavp.tile([128, KT, N], F32, tag="pa")
                    for s in range(H):
                        h = 2 * (s % 6) + (s // 6)  # head held in ES slot s
                        nc.tensor.matmul(
                            pa[64 * (s // 6):64 * (s // 6) + 64, s % 6, :],
                            mm(V[:, w, 64 * h:64 * (h + 1)]),
                            mm(ESN[:, s, :]),
                            start=True, stop=True, skip_group_check=True,
                        )
                    nc.vector.tensor_copy(AT[:, :, w, :], pa[:])

                esns = {0: qk_softmax(0)}
                for w in range(1, G):
                    esns[w] = qk_softmax(w)
                    av(w - 1, esns.pop(w - 1))
                av(G - 1, esns.pop(G - 1))

                if DEBUG_TAPS and g == 0:
                    nc.sync.dma_start(dbg["at"].ap(), AT[:])
                # ---- proj: out^T[o, w, t] straight from PSUM to DRAM
                for j in range(KT):
                    po = pbig.tile([128, FD], F32, tag="big")
                    for kt in range(KT):
                        nc.tensor.matmul(
                            po[:], mm(WPs[:, j, kt, :]), mm(AT[:, kt, :, :]),
                            start=(kt == 0), stop=False,
                        )
                    nc.tensor.matmul(
                        po[:], mm(BPs[:, 128 * j:128 * (j + 1)]), mm(ONES1s[:]),
                        start=False, stop=True,
                    )
                    OT = atp.tile([128, G, N], F32, tag="ot")
                    src = po.rearrange("p (a b) -> p a b", a=G)
                    if j % 2 == 0:
                        nc.vector.tensor_copy(OT[:], src)
                    else:
                        nc.scalar.copy(OT[:], src)
                    nc.sync.dma_start(out_d.ap()[:, j, gsl, :], OT[:])

    nc.compile()
    return nc


def _host_prep(x, pe, w_qkv, b_qkv, w_proj, b_proj):
    f = np.float32
    x = np.asarray(x, f)
    pe = np.asarray(pe, f)
    w_qkv = np.asarray(w_qkv, f)
    b_qkv = np.asarray(b_qkv, f)
    w_proj = np.asarray(w_proj, f)
    b_proj = np.asarray(b_proj, f)

    scale = f(HD ** -0.5)
    n_ = N - N_VTS
    strt = pe.shape[0] // 2 - n_ // 2
    peX = np.zeros((N, C), f)
    peX[N_VTS:] = pe[strt:strt + n_]
    pe_qkv = (peX @ w_qkv.T + b_qkv).astype(f)     # (68, 2304)
    pe_qkv[:, :C] *= scale

    w_qk = np.concatenate([w_qkv[:C] * scale, w_qkv[C:2 * C]], axis=0)  # (1536, 768)
    W1 = np.ascontiguousarray(
        w_qk.reshape(12, 128, KT, 128).transpose(3, 0, 2, 1))           # [p,j,k,o]
    W2 = np.ascontiguousarray(
        w_qkv[2 * C:].reshape(C, KT, 128).transpose(2, 1, 0))           # [p,k,o]
    WP = np.ascontiguousarray(
        w_proj.reshape(KT, 128, KT, 128).transpose(3, 0, 2, 1))         # [p,j,kt,o]
    PEQK = np.ascontiguousarray(pe_qkv[:, :2 * C].reshape(N, 12, 128))
    PEV = np.ascontiguousarray(pe_qkv[:, 2 * C:])
    BP = b_proj.reshape(1, C).copy()
    I68R = np.ascontiguousarray(np.tile(np.eye(N, dtype=f), (1, G)))
    ONES1 = np.ones((1, FD), f)
    ONES68 = np.ones((N, 1), f)

    # x (1024, 68, 768) -> [core, p, k, b_local, t]
    xt = np.ascontiguousarray(
        x.reshape(NCORES, BL, N, KT, 128).transpose(0, 4, 3, 1, 2))

    shared = {
        "w1": W1, "w2": W2, "wp": WP, "peqk": PEQK, "pev": PEV, "bp": BP,
        "i68r": I68R, "ones1": ONES1, "ones68": ONES68,
    }
    return xt, shared


def kernel(x, pe, mask, w_qkv, b_qkv, w_proj, b_proj):
    del mask  # zeros by problem spec
    xt, shared = _host_prep(x, pe, w_qkv, b_qkv, w_proj, b_proj)

    mm_dt = _CACHE.get("mm_dt", mybir.dt.float32r)
    if "nc" not in _CACHE:
        _CACHE["nc"] = _build_nc(mm_dt)
    nc = _CACHE["nc"]

    in_maps = [dict(shared, xt=xt[c]) for c in range(NCORES)]
    res = run_bass_kernel_spmd(
        nc, in_maps, core_ids=list(range(NCORES)),
        **_CACHE.get("run_kwargs", {}),
    )
    _CACHE["last_result"] = res

    # outt [core, p, j, b, t] -> (1024, 68, 768)
    outt = np.stack([res.results[c]["outt"] for c in range(NCORES)])
    out = np.ascontiguousarray(
        outt.transpose(0, 3, 4, 2, 1).reshape(B_, N, C))
    return out



# revision 7
# speedup vs baseline: 1.6061x; 1.6061x over previous
"""Trainium2 Bass kernel for windowed multi-head attention (ClassicAttention).

Shapes (hardcoded per spec): x (1024, 68, 768), pe (128, 768), mask zeros.
Data-parallel over 8 NeuronCores on the leading window axis.

v2: bf16 matmul operands throughout (fp32 moving operands cost 2-4
cycles/row on the PE; bf16 costs 1), pe folded into x on the host,
biases folded into PSUM->SBUF copies / proj bias, softmax normalization
applied post-AV via a broadcast multiply (drops the ESN multiply and
the slow single-lane reciprocal in the critical chain;
reciprocal_approx_fast is ~5x faster and accurate to ~18 bits).
"""

import os
import sys

for _p in (
    "/root/.axon_site",
    "/root/.axon_site/_ro/trn_rl_repo",
    "/root/.axon_site/_ro/pypackages",
    "/opt/trn_rl_repo",
):
    if os.path.isdir(_p) and _p not in sys.path:
        sys.path.append(_p)

import ml_dtypes
import numpy as np

import concourse.bass as bass
import concourse.mybir as mybir
import concourse.tile as tile
from concourse import bacc
from concourse.bass_utils import run_bass_kernel_spmd

F32 = mybir.dt.float32
BF16 = mybir.dt.bfloat16
AFT = mybir.ActivationFunctionType

NCORES = 8
B_, N, C = 1024, 68, 768
H, HD = 12, 64
N_VTS = 4
KT = C // 128            # 6 contraction tiles of 128
BL = B_ // NCORES        # 128 windows per core
G = 4                    # windows per group
NG = BL // G             # 32 groups
FD = G * N               # 272

_CACHE = {}


def _build_nc():
    nc = bacc.Bacc(trn_type="TRN2", target_bir_lowering=False, debug=False)

    xt_d = nc.dram_tensor("xt", [128, KT, BL, N], BF16, kind="ExternalInput")
    w1_d = nc.dram_tensor("w1", [128, 12, KT, 128], BF16, kind="ExternalInput")
    w2_d = nc.dram_tensor("w2", [128, KT, C], BF16, kind="ExternalInput")
    wp_d = nc.dram_tensor("wp", [128, KT, KT, 128], BF16, kind="ExternalInput")
    bqk_d = nc.dram_tensor("bqk", [128, 12], F32, kind="ExternalInput")
    bpp_d = nc.dram_tensor("bpp", [128, KT], F32, kind="ExternalInput")
    ones68_d = nc.dram_tensor("ones68", [N, 1], BF16, kind="ExternalInput")
    out_d = nc.dram_tensor("outt", [128, KT, BL, N], F32, kind="ExternalOutput")

    with tile.TileContext(nc) as tc:
        with (
            tc.tile_pool(name="wgt", bufs=1) as wp_pool,
            tc.tile_pool(name="xp", bufs=2) as xp,
            tc.tile_pool(name="qkp", bufs=2) as qkp,
            tc.tile_pool(name="vp", bufs=2) as vp,
            tc.tile_pool(name="esp", bufs=6) as esp,
            tc.tile_pool(name="atp", bufs=2) as atp,
            tc.tile_pool(name="rp", bufs=2) as rp,
            tc.tile_pool(name="rbp", bufs=6) as rbp,
            tc.tile_pool(name="otp", bufs=3) as otp,
            tc.tile_pool(name="pbig", bufs=2, space="PSUM") as pbig,
            tc.tile_pool(name="ppv", bufs=1, space="PSUM") as ppv,
            tc.tile_pool(name="psc", bufs=2, space="PSUM") as psc,
            tc.tile_pool(name="ps1", bufs=1, space="PSUM") as ps1p,
            tc.tile_pool(name="pav", bufs=1, space="PSUM") as pavp,
        ):
            W1s = wp_pool.tile([128, 12, KT, 128], BF16)
            W2s = wp_pool.tile([128, KT, C], BF16)
            WPs = wp_pool.tile([128, KT, KT, 128], BF16)
            BQKs = wp_pool.tile([128, 12], F32)
            BPPs = wp_pool.tile([128, KT], F32)
            ONES68s = wp_pool.tile([N, 1], BF16)
            nc.sync.dma_start(W1s[:], w1_d.ap())
            nc.sync.dma_start(W2s[:], w2_d.ap())
            nc.sync.dma_start(WPs[:], wp_d.ap())
            nc.sync.dma_start(BQKs[:], bqk_d.ap())
            nc.sync.dma_start(BPPs[:], bpp_d.ap())
            nc.sync.dma_start(ONES68s[:], ones68_d.ap())

            for g in range(NG):
                gsl = slice(G * g, G * (g + 1))
                XT = xp.tile([128, KT, G, N], BF16, tag="xt")
                nc.gpsimd.dma_start(XT[:], xt_d.ap()[:, :, gsl, :])

                # ---- q,k in transposed layout: QKT[p, j, w, t] (j<6: q, j>=6: k)
                QKT = qkp.tile([128, 12, G, N], BF16, tag="qkt")
                for j in range(12):
                    pq = pbig.tile([128, FD], F32, tag="big")
                    for k in range(KT):
                        nc.tensor.matmul(
                            pq[:], W1s[:, j, k, :], XT[:, k, :, :],
                            start=(k == 0), stop=(k == KT - 1),
                        )
                    nc.scalar.activation(
                        QKT[:, j, :, :], pq.rearrange("p (a b) -> p a b", a=G),
                        AFT.Identity, bias=BQKs[:, j:j + 1],
                    )

                # ---- attention scores + exp + sums, per window
                ES = {}
                S1 = {}

                def qk_exp(w):
                    # ES slot s = 6*half + hh holds head h = 2*hh + half, so
                    # each PSUM bank sees a single PE row-group (HW hangs on
                    # mixed-row-group matmuls into one bank).
                    ES[w] = esp.tile([N, H, N], BF16, tag="es", name="es")
                    for half in range(2):
                        sc = psc.tile([N, 6, N], F32, tag="sc")
                        p0 = 64 * half
                        for hh in range(6):
                            nc.tensor.matmul(
                                sc[:, hh, :],
                                QKT[p0:p0 + 64, 6 + hh, w, :],
                                QKT[p0:p0 + 64, hh, w, :],
                                start=True, stop=True, skip_group_check=True,
                            )
                        nc.scalar.activation(
                            ES[w][:, 6 * half:6 * half + 6, :], sc[:], AFT.Exp
                        )

                R = {}

                def sums(w):
                    # both halves' sums at partition 0 (custom-DVE ops require
                    # partition-0 APs); the 512-pad keeps each half's 408-col
                    # matmul output inside a single PSUM bank
                    S1[w] = ps1p.tile([1, 2, 512], F32, tag="s1", name="s1")
                    for half in range(2):
                        nc.tensor.matmul(
                            S1[w][0:1, half, 0:6 * N],
                            ONES68s[:],
                            ES[w][:, 6 * half:6 * half + 6, :],
                            start=True, stop=True, skip_group_check=True,
                        )
                    # 1/rowsum immediately (vector), so the ps1 PSUM banks
                    # recycle without stalling the tensor queue, then
                    # broadcast to all 128 partitions for the post-AV scale
                    r1 = rp.tile([1, 2, 6 * N], F32, tag="r1")
                    for half in range(2):
                        nc.vector.reciprocal_approx_fast(
                            r1[0:1, half, :],
                            S1[w][0:1, half, 0:6 * N],
                        )
                    R[w] = rbp.tile([128, 6, N], F32, tag="rb", name="rb")
                    for half in range(2):
                        row = r1[0:1, half, :]
                        src = bass.AP(
                            tensor=row.tensor, offset=row.offset,
                            ap=[list(row.ap[0]), [0, 64], [1, 6 * N]],
                        )
                        nc.sync.dma_start(R[w][64 * half:64 * half + 64, :, :], src)

                for w in range(G):
                    qk_exp(w)
                    if w >= 1:
                        sums(w - 1)
                sums(G - 1)

                # ---- v in direct layout: V[t, w, o] (one PSUM bank, two
                # sequential half-GEMMs per window)
                V = vp.tile([N, G, C], BF16, tag="v")
                for w in range(G):
                    for hs in (slice(0, 512), slice(512, C)):
                        pv = ppv.tile([N, 512], F32, tag="pv")
                        hw_ = hs.stop - hs.start
                        for k in range(KT):
                            nc.tensor.matmul(
                                pv[:, 0:hw_], XT[:, k, w, :], W2s[:, k, hs],
                                start=(k == 0), stop=(k == KT - 1),
                            )
                        nc.vector.tensor_copy(V[:, w, hs], pv[:, 0:hw_])

                # ---- AV into AT[o, kt, w, t], normalized during PSUM drain
                AT = atp.tile([128, KT, G, N], BF16, tag="at")
                for w in range(G):
                    pa = pavp.tile([128, KT, N], F32, tag="pa")
                    for s in range(H):
                        h = 2 * (s % 6) + (s // 6)  # head held in ES slot s
                        nc.tensor.matmul(
                            pa[64 * (s // 6):64 * (s // 6) + 64, s % 6, :],
                            V[:, w, 64 * h:64 * (h + 1)],
                            ES[w][:, s, :],
                            start=True, stop=True, skip_group_check=True,
                        )
                    nc.vector.tensor_mul(AT[:, :, w, :], pa[:], R[w][:])

                # ---- proj: out^T[o, w, t] with bias folded into the drain
                for j in range(KT):
                    po = pbig.tile([128, FD], F32, tag="big")
                    for kt in range(KT):
                        nc.tensor.matmul(
                            po[:], WPs[:, j, kt, :], AT[:, kt, :, :],
                            start=(kt == 0), stop=(kt == KT - 1),
                        )
                    OT = otp.tile([128, G, N], F32, tag="ot")
                    nc.scalar.activation(
                        OT[:], po.rearrange("p (a b) -> p a b", a=G),
                        AFT.Identity, bias=BPPs[:, j:j + 1],
                    )
                    nc.sync.dma_start(out_d.ap()[:, j, gsl, :], OT[:])

    nc.compile()
    return nc


def _host_prep(x, pe, w_qkv, b_qkv, w_proj, b_proj):
    f = np.float32
    bf = ml_dtypes.bfloat16
    x = np.asarray(x, f)
    pe = np.asarray(pe, f)
    w_qkv = np.asarray(w_qkv, f)
    b_qkv = np.asarray(b_qkv, f)
    w_proj = np.asarray(w_proj, f)
    b_proj = np.asarray(b_proj, f)

    scale = f(HD ** -0.5)
    n_ = N - N_VTS
    strt = pe.shape[0] // 2 - n_ // 2

    # fold pe into x on the host; shard and transpose to [p, k, b, t]
    xp = x.copy()
    xp[:, N_VTS:, :] += pe[strt:strt + n_]
    xt = np.ascontiguousarray(
        xp.reshape(NCORES, BL, N, KT, 128).transpose(0, 4, 3, 1, 2)).astype(bf)

    w_qk = np.concatenate([w_qkv[:C] * scale, w_qkv[C:2 * C]], axis=0)  # (1536, 768)
    W1 = np.ascontiguousarray(
        w_qk.reshape(12, 128, KT, 128).transpose(3, 0, 2, 1)).astype(bf)  # [p,j,k,o]
    W2 = np.ascontiguousarray(
        w_qkv[2 * C:].reshape(C, KT, 128).transpose(2, 1, 0)).astype(bf)  # [p,k,o]
    WP = np.ascontiguousarray(
        w_proj.reshape(KT, 128, KT, 128).transpose(3, 0, 2, 1)).astype(bf)  # [p,j,kt,o]

    b_qk = np.concatenate([b_qkv[:C] * scale, b_qkv[C:2 * C]])
    BQK = np.ascontiguousarray(b_qk.reshape(12, 128).T).astype(f)        # [p, j]
    # b_v folds into the proj bias: softmax rows sum to 1, so
    # proj(attn @ (v + b_v)) = proj(attn @ v) + w_proj @ b_v
    bpp = b_proj + w_proj @ b_qkv[2 * C:]
    BPP = np.ascontiguousarray(bpp.reshape(KT, 128).T).astype(f)         # [p, j]
    ONES68 = np.ones((N, 1), bf)

    shared = {
        "w1": W1, "w2": W2, "wp": WP, "bqk": BQK, "bpp": BPP,
        "ones68": ONES68,
    }
    return xt, shared


def kernel(x, pe, mask, w_qkv, b_qkv, w_proj, b_proj):
    del mask  # zeros by problem spec
    xt, shared = _host_prep(x, pe, w_qkv, b_qkv, w_proj, b_proj)

    if "nc" not in _CACHE:
        _CACHE["nc"] = _build_nc()
    nc = _CACHE["nc"]

    in_maps = [dict(shared, xt=xt[c]) for c in range(NCORES)]
    res = run_bass_kernel_spmd(
        nc, in_maps, core_ids=list(range(NCORES)),
        **_CACHE.get("run_kwargs", {}),
    )
    _CACHE["last_result"] = res

    # outt [core, p, j, b, t] -> (1024, 68, 768)
    outt = np.stack([res.results[c]["outt"] for c in range(NCORES)])
    out = np.ascontiguousarray(
        outt.transpose(0, 3, 4, 2, 1).reshape(B_, N, C))
    return out
